# revision 1
# baseline (speedup 1.0000x reference)
"""Trainium2 Bass kernel for nn_CNN_V1_32796370272431.

Math (see reference):
    h   = relu(const_vec @ W1^T + b1)          # [F, HID]       tiny
    k1  = einsum('fh,fsh->fs', h, W2) + b2     # [F, S]         tiny
    k2  = k1 @ smooth                          # [F, S]         tiny
    outs= einsum('bsf,fs->bf', x, k2)          # [B, F]         268MB of x -> memory bound
    out = relu(outs @ fcW1.T + fcb1) @ fcW2.T + fcb2   # [B, 1] tiny

Everything except the big contraction depends only on the small weight
tensors, so k2 and the fc weights are folded on the host.  The device
kernel streams x at HBM rate and computes, per batch row b:

    out[b, f] = sum_s x[b, s, f] * k2[f, s]

Layout trick: x[b] is a contiguous 1MB block [S=4096, F=64].  Loaded as a
flat SBUF tile [128, 2048] (partition p holds linear elements
[p*2048, (p+1)*2048) = s in [p*32, (p+1)*32) x all f), the DMA is
perfectly contiguous (8KB runs per partition).  k2^T reshaped the same
way ("m" tile) lets one DVE tensor_tensor multiply produce
tmp[p, j] = x*k2 in matching layout.  The partition-axis reduction is a
TensorE matmul with a one-hot stationary matrix E_b (ones in column b),
accumulating every batch row's partial sums into PSUM row b:

    P[b, j] += sum_p tmp_b[p, c*512 + j]        (c = 4 chunks of 512)

leaving an 8-way strided free-axis sum (P viewed as [32, 8, 64]) for one
DVE tensor_reduce -> outs [32 b, 64 f].  The fc head runs transposed on
TensorE (outs^T via PE transpose) so biases become per-partition ACT
biases.  Each core handles 32 batch rows; output is [1, 32] per core.

The reduction matmuls run as float32r (single-pass fp32, ~tf32
precision, 1 cycle/row at N=512); the elementwise multiply and
everything else stay full fp32.
"""

import numpy as np

import concourse.bass as bass
import concourse.mybir as mybir
from concourse.bass_utils import run_bass_kernel_spmd
from concourse.tile import TileContext

# Problem constants (hardcoded per harness contract).
B, S, F, HID = 256, 4096, 64, 10
N_CORES = 8
NB = B // N_CORES            # batch rows per core = 32
BPD = 2                      # batch rows per DMA tile
NT = NB // BPD               # x tiles per core = 16
COLS = S * F // 128          # 2048 free columns of a flat per-row tile
NCH = COLS // 512            # 512-wide matmul chunks per row = 4
SL = COLS // F               # s-positions folded per psum column group = 8

F32 = mybir.dt.float32
F32R = mybir.dt.float32r
F16 = mybir.dt.float16

_PROGRAM_CACHE = {}
# Stream x/k2 as fp16 (half the HBM bytes, ~2x faster; adds ~4e-4 rel err
# on top of f32 — PSUM accumulation and the fc head remain fp32).
HALF = True


def _split_excess_waits(nc):
    """Walrus (this build) accepts at most one sync-wait per instruction
    (two on InstEventSemaphore), but the Tile scheduler can attach more.
    Move the excess onto same-engine InstNoOps placed immediately before
    the instruction — identical semantics, since the engine sequencer
    executes its stream in order."""
    for fn in nc.m.functions:
        for bb in fn.blocks:
            out = []
            changed = False
            for ins in bb.instructions:
                si = ins.sync_info
                cap = 2 if isinstance(ins, mybir.InstEventSemaphore) else 1
                if si is not None and si.on_wait and len(si.on_wait) > cap:
                    waits = list(si.on_wait)
                    for w in waits[:-cap]:
                        nop = mybir.InstNoOp(
                            name=nc.get_next_instruction_name(),
                            engine=ins.engine,
                            bass_nofuse=True,
                            sync_info=mybir.SyncInfo(on_wait=[w], on_update=[]),
                        )
                        nc.register_instruction(nop, overwrite=True)
                        out.append(nop)
                    si.on_wait = waits[-cap:]
                    changed = True
                out.append(ins)
            if changed:
                bb.instructions = out


def _build_program(
    reps=1, loop_iters=0, bpd=BPD, xbufs=3, tbufs=8, dual_ring=False,
    skip_compute=False, skip_dma=False, inplace=False, half=False, hl=False,
):
    """Build the (SPMD, per-core) bass program once; inputs are DRAM params.

    reps > 1 repeats the full streaming loop (for benchmarking: the
    marginal wall time per extra rep is the steady-state kernel time,
    free of dispatch/transfer overhead).  loop_iters > 0 additionally
    wraps the reps bodies in a hardware For_i loop (loop_iters * reps
    total passes) so kernel time can dominate per-call dispatch."""
    nc = bass.Bass(trn_type="TRN2", target_bir_lowering=False)

    # half=True streams x (and the k2 tile) as fp16: halves HBM traffic and
    # unlocks the DVE 2x mode; PSUM accumulation and the fc head stay fp32.
    # With inplace=True the DVE multiply overwrites the x tile, whose dtype
    # must then be float32r so the FP32r matmuls may consume it (same bits).
    x_dt = F16 if half else (F32R if inplace else F32)
    md_dt = F16 if half else F32
    e_dt = F16 if half else F32R
    # hl=True: host pre-interleaves the b-rows of each DMA tile so the DRAM
    # source of every x load is one contiguous [128, bpd*COLS] block
    # (bpd*COLS*dtype bytes per partition, single linear run).
    if hl:
        x_d = nc.declare_dram_parameter(
            "x", [NB // bpd, 128, bpd * COLS], x_dt, isOutput=False)
    else:
        x_d = nc.declare_dram_parameter("x", [NB, 128, COLS], x_dt, isOutput=False)
    m_d = nc.declare_dram_parameter("m", [128, COLS], md_dt, isOutput=False)
    e_d = nc.declare_dram_parameter("eye32", [128, NB * 32], e_dt, isOutput=False)
    w1_d = nc.declare_dram_parameter("fcW1T", [F, HID], F32, isOutput=False)
    b1_d = nc.declare_dram_parameter("fcb1", [HID, 1], F32, isOutput=False)
    w2_d = nc.declare_dram_parameter("fcW2T", [HID, 1], F32, isOutput=False)
    b2_d = nc.declare_dram_parameter("fcb2", [1, 1], F32, isOutput=False)
    id_d = nc.declare_dram_parameter("ident", [32, 32], F32, isOutput=False)
    out_d = nc.declare_dram_parameter("out", [1, NB], F32, isOutput=True)

    with TileContext(nc) as tc:
        with (
            tc.tile_pool(name="const", bufs=1) as cpool,
            tc.tile_pool(name="xin", bufs=xbufs) as xpool,
            tc.tile_pool(name="tmp", bufs=tbufs) as tpool,
            tc.tile_pool(name="small", bufs=1) as spool,
            tc.tile_pool(name="acc", bufs=1, space="PSUM") as apool,
            tc.tile_pool(name="ptail", bufs=1, space="PSUM") as ppool,
        ):
            m_sb = cpool.tile([128, COLS], md_dt)
            e_sb = cpool.tile([128, NB * 32], e_dt)
            w1_sb = cpool.tile([F, HID], F32)
            b1_sb = cpool.tile([HID, 1], F32)
            w2_sb = cpool.tile([HID, 1], F32)
            b2_sb = cpool.tile([1, 1], F32)
            id_sb = cpool.tile([32, 32], F32)
            # Const loads on the ACT HWDGE ring so they overlap with the
            # x stream on the SP ring from the very first instruction.
            nc.scalar.dma_start(out=m_sb[:], in_=m_d[:])
            nc.scalar.dma_start(out=e_sb[:], in_=e_d[:])
            nc.scalar.dma_start(out=w1_sb[:], in_=w1_d[:])
            nc.scalar.dma_start(out=b1_sb[:], in_=b1_d[:])
            nc.scalar.dma_start(out=w2_sb[:], in_=w2_d[:])
            nc.scalar.dma_start(out=b2_sb[:], in_=b2_d[:])
            nc.scalar.dma_start(out=id_sb[:], in_=id_d[:])

            acc = apool.tile([NB, 512], F32)  # one PSUM bank, row b = batch b

            xt_static = None
            if skip_dma:
                xt_static = cpool.tile([128, bpd * COLS], F32)
                nc.sync.dma_start(
                    out=xt_static[:].rearrange("p (b j) -> p b j", b=bpd),
                    in_=x_d[0:bpd].rearrange("b p j -> p b j"),
                )

            def _bodies():
                for _rep in range(reps):
                    _main_loop_and_tail(
                        nc, x_d, out_d, m_sb, e_sb, w1_sb, b1_sb, w2_sb, b2_sb,
                        id_sb, acc, xpool, tpool, spool, ppool,
                        bpd=bpd, dual_ring=dual_ring,
                        skip_compute=skip_compute, xt_static=xt_static,
                        inplace=inplace, half=half, hl=hl,
                    )

            if loop_iters:
                hints = (
                    mybir.EngineType.PE,
                    mybir.EngineType.DVE,
                    mybir.EngineType.SP,
                    mybir.EngineType.Activation,
                )
                with tc.For_i(0, loop_iters, 1, hint_engines=hints):
                    _bodies()
            else:
                _bodies()

    _split_excess_waits(nc)
    return nc


def _main_loop_and_tail(
    nc, x_d, out_d, m_sb, e_sb, w1_sb, b1_sb, w2_sb, b2_sb,
    id_sb, acc, xpool, tpool, spool, ppool,
    bpd=BPD, dual_ring=False, skip_compute=False, xt_static=None,
    inplace=False, half=False, hl=False,
):
    if True:
        if True:
            x_dt = F16 if half else (F32R if inplace else F32)
            nt = NB // bpd
            n_mm = NB * NCH
            mm = 0
            for t in range(nt):
                if xt_static is None:
                    xt = xpool.tile([128, bpd * COLS], x_dt)
                    dma_eng = nc.scalar if (dual_ring and t % 2) else nc.sync
                    if hl:
                        dma_eng.dma_start(out=xt[:], in_=x_d[t])
                    else:
                        dma_eng.dma_start(
                            out=xt[:].rearrange("p (b j) -> p b j", b=bpd),
                            in_=x_d[t * bpd : (t + 1) * bpd].rearrange("b p j -> p b j"),
                        )
                else:
                    xt = xt_static
                if skip_compute:
                    xt  # DMA-only probe: no consumers
                    continue
                for i in range(bpd):
                    b = t * bpd + i
                    if inplace:
                        tt = xt[:, i * COLS : (i + 1) * COLS]
                        nc.vector.tensor_mul(out=tt, in0=tt, in1=m_sb[:])
                    else:
                        tt_t = tpool.tile([128, COLS], F16 if half else F32R)
                        tt = tt_t[:]
                        nc.vector.tensor_mul(
                            out=tt,
                            in0=xt[:, i * COLS : (i + 1) * COLS],
                            in1=m_sb[:],
                        )
                    for c in range(NCH):
                        nc.tensor.matmul(
                            out=acc[:],
                            lhsT=e_sb[:, b * 32 : (b + 1) * 32],
                            rhs=tt[:, c * 512 : (c + 1) * 512],
                            start=(mm == 0),
                            stop=(mm == n_mm - 1),
                        )
                        mm += 1

            if skip_compute:
                out_sb = spool.tile([1, NB], F32)
                nc.vector.tensor_copy(out=out_sb[:], in_=m_sb[0:1, 0:NB])
                nc.sync.dma_start(out=out_d[:], in_=out_sb[:])
                return

            # acc[b, j] with j = s_lo*64 + f  ->  outs[b, f] = sum_{s_lo}
            outs_sb = spool.tile([NB, F], F32)
            nc.vector.tensor_reduce(
                out=outs_sb[:],
                in_=acc[:].rearrange("b (s f) -> b f s", f=F),
                axis=mybir.AxisListType.X,
                op=mybir.AluOpType.add,
            )

            # fc head, transposed: outsT = PE-transpose(outs) -> [F, NB]
            outsT_ps = ppool.tile([F, NB], F32)
            nc.tensor.transpose(out=outsT_ps[:], in_=outs_sb[:], identity=id_sb[:])
            outsT_sb = spool.tile([F, NB], F32)
            nc.vector.tensor_copy(out=outsT_sb[:], in_=outsT_ps[:])

            hh_ps = ppool.tile([HID, NB], F32)
            nc.tensor.matmul(
                out=hh_ps[:], lhsT=w1_sb[:], rhs=outsT_sb[:], start=True, stop=True
            )
            hhT_sb = spool.tile([HID, NB], F32)
            nc.scalar.activation(
                out=hhT_sb[:],
                in_=hh_ps[:],
                func=mybir.ActivationFunctionType.Relu,
                bias=b1_sb[:],
            )

            f_ps = ppool.tile([1, NB], F32)
            nc.tensor.matmul(
                out=f_ps[:], lhsT=w2_sb[:], rhs=hhT_sb[:], start=True, stop=True
            )
            out_sb = spool.tile([1, NB], F32)
            nc.scalar.activation(
                out=out_sb[:],
                in_=f_ps[:],
                func=mybir.ActivationFunctionType.Identity,
                bias=b2_sb[:],
            )
            nc.sync.dma_start(out=out_d[:], in_=out_sb[:])


def _host_weights(W1, b1, W2, b2, fcW1, fcb1, fcW2, fcb2, const_vec, smooth,
                  half=False):
    """Fold the tiny weight tensors into the device-side constants."""
    h = np.maximum(np.einsum("c,fhc->fh", const_vec, W1) + b1, 0.0)
    k1 = np.einsum("fh,fsh->fs", h.astype(np.float32), W2) + b2
    k2 = (k1.astype(np.float32) @ smooth).astype(np.float32)  # [F, S]
    hdt = np.float16 if half else np.float32
    m_flat = np.ascontiguousarray(k2.T.reshape(128, COLS), dtype=hdt)

    eye32 = np.zeros((128, NB * 32), dtype=hdt)
    for b in range(NB):
        eye32[:, b * 32 + b] = 1.0

    return {
        "m": m_flat,
        "eye32": eye32,
        "fcW1T": np.ascontiguousarray(fcW1.T, dtype=np.float32),
        "fcb1": np.ascontiguousarray(fcb1.reshape(HID, 1), dtype=np.float32),
        "fcW2T": np.ascontiguousarray(fcW2.T, dtype=np.float32),
        "fcb2": np.ascontiguousarray(np.reshape(fcb2, (1, 1)), dtype=np.float32),
        "ident": np.eye(32, dtype=np.float32),
    }


def _enable_jit_cache():
    try:
        import jax

        jax.config.update("jax_compilation_cache_dir", "/tmp/jax_bass_cache")
        jax.config.update("jax_persistent_cache_min_entry_size_bytes", -1)
        jax.config.update("jax_persistent_cache_min_compile_time_secs", 0.5)
    except Exception:
        pass


def run(inputs, trace=False, reps=1, half=HALF, **run_kwargs):
    """Run on 8 NeuronCores; returns (full_output, BassKernelResults)."""
    _enable_jit_cache()
    key = ("prog", reps, half)
    if key not in _PROGRAM_CACHE:
        # fp16 rows are 4KB; group 4 per DMA tile (-3.4us vs bpd=2) and
        # pre-interleave on the host so each x load is one fully linear
        # 2MB DRAM block (-1us).
        # dual_ring alternates x loads across both HWDGE rings: hides the
        # per-chunk completion tails, which matter at fp16 chunk counts
        # (-1.8us measured).
        _PROGRAM_CACHE[key] = _build_program(
            reps=reps, half=half,
            bpd=(4 if half else BPD), xbufs=(4 if half else 3), hl=half,
            dual_ring=half,
        )
    nc = _PROGRAM_CACHE[key]

    xdt = np.float16 if half else np.float32
    x = np.ascontiguousarray(np.asarray(inputs["x"]).astype(xdt))
    consts = _host_weights(
        *(
            np.asarray(inputs[k], dtype=np.float32)
            for k in (
                "W1", "b1", "W2", "b2",
                "fcW1", "fcb1", "fcW2", "fcb2",
                "const_vec", "smooth",
            )
        ),
        half=half,
    )

    core_ids = list(range(N_CORES))
    in_maps = []
    for c in core_ids:
        sh = x[c * NB : (c + 1) * NB]
        if half:
            # match hl=True: [NT, 128, bpd*COLS] with b-rows interleaved
            shard = np.ascontiguousarray(
                sh.reshape(NB // 4, 4, 128, COLS)
                .transpose(0, 2, 1, 3)
                .reshape(NB // 4, 128, 4 * COLS)
            )
        else:
            shard = sh.reshape(NB, 128, COLS)
        in_maps.append({"x": shard, **consts})

    res = run_bass_kernel_spmd(nc, in_maps, core_ids, trace=trace, **run_kwargs)
    out = np.concatenate(
        [np.asarray(res.results[c]["out"]).reshape(NB) for c in core_ids]
    )
    return out.reshape(B, 1).astype(np.float32), res


def kernel(**inputs) -> np.ndarray:
    out, _ = run(inputs)
    return out

